# revision 1
# baseline (speedup 1.0000x reference)
"""Trainium2 Bass kernel for nn_DepthCue (dynamic-filter / CARAFE-style module).

Sharding: data-parallel over batch B=8 across the 8 NeuronCores (one sample
per core). Per core:
  - guide network (3x3 convs C->64->64->C) + DCK (1x1 convs + BN/ReLU) run on
    TensorE as shifted matmuls in float32r (1 cyc/row at N=512).
  - dynamic-filter apply: partitions = (h-band, group); per-tap elementwise
    multiply on VectorE in bf16 (filters broadcast across the 16 group
    channels via a step-0 AP dim); tap accumulation via identity-matmul into
    PSUM (fp32); residual added via an f32r identity-matmul PSUM init.
"""

import numpy as np
import ml_dtypes

import concourse.bass as bass
import concourse.bacc as bacc
import concourse.mybir as mybir
from concourse import bass_utils
from concourse.tile import TileContext

F32 = mybir.dt.float32
F32R = mybir.dt.float32r
BF16 = mybir.dt.bfloat16
MULT = mybir.AluOpType.mult
RELU = mybir.ActivationFunctionType.Relu
COPY = mybir.ActivationFunctionType.Copy

N_CORES = 8
C, H, W = 512, 64, 64
HID = 64          # guide-net hidden channels
RED = 128         # DCK reduction channels
G = 32            # groups
GC = 16           # channels per group
K = 7             # dynamic kernel size
NTAP = K * K      # 49
TPAD = 64         # taps padded to 64 in dck2 weights / fbuf
NB = 4            # h-bands (partition index = band*32 + g)
BH = 16           # rows per band
XH = BH + 6       # xb stored rows per band (halo 3 top + 3 bottom)
XW = W + 6        # xb stored cols (halo 3 + 3)
CST = XH * XW     # per-channel stride in xb free dim (1540)
PIX = H * W       # 4096
PW = W + 2        # padded width for conv intermediates (66)
PHW = (H + 2) * PW


def ap_of(t, offset, dims):
    """Raw AP over tile/dram tensor t: dims = [[step, count], ...] (dim0 = partition for sbuf)."""
    base = t if isinstance(t, bass.AP) else t[:]
    return bass.AP(tensor=base.tensor, offset=offset, ap=[list(d) for d in dims])


def build_nc():
    nc = bacc.Bacc(trn_type="TRN2", target_bir_lowering=False, debug=False)

    T = {}
    T["x_in"] = nc.dram_tensor("x", [C, H, W], F32, kind="ExternalInput").ap()
    T["xr"] = nc.dram_tensor("xr", [C, H, W], F32R, kind="ExternalInput").ap()
    T["w1t"] = nc.dram_tensor("w1t", [128, 9 * 4 * HID], F32R, kind="ExternalInput").ap()
    T["b1"] = nc.dram_tensor("b1", [HID, 1], F32, kind="ExternalInput").ap()
    T["w2t"] = nc.dram_tensor("w2t", [HID, 9 * HID], F32R, kind="ExternalInput").ap()
    T["b2"] = nc.dram_tensor("b2", [HID, 1], F32, kind="ExternalInput").ap()
    T["w3t"] = nc.dram_tensor("w3t", [HID, 9 * C], F32R, kind="ExternalInput").ap()
    T["b3"] = nc.dram_tensor("b3", [128, 4], F32, kind="ExternalInput").ap()
    T["dw1t"] = nc.dram_tensor("dw1t", [128, 4 * RED], F32R, kind="ExternalInput").ap()
    T["bnsc"] = nc.dram_tensor("bnsc", [RED, 1], F32, kind="ExternalInput").ap()
    T["bnsh"] = nc.dram_tensor("bnsh", [RED, 1], F32, kind="ExternalInput").ap()
    T["dw2t"] = nc.dram_tensor("dw2t", [RED, G * TPAD], F32R, kind="ExternalInput").ap()
    T["idb"] = nc.dram_tensor("idb", [128, 128], BF16, kind="ExternalInput").ap()
    T["out"] = nc.dram_tensor("out", [C, H, W], F32, kind="ExternalOutput").ap()
    # filters scratch: [band, g, tap(64), h(16), w] bf16
    T["fbuf"] = nc.dram_tensor("fbuf", [NB, G, TPAD, BH, W], BF16, kind="Internal").ap()

    with TileContext(nc) as tc:
        build_body(nc, tc, T)
    nc.compile()
    return nc


def conv_rhs(src, r0, tap, nh):
    dy, dx = tap // 3, tap % 3
    return ap_of(
        src, (r0 + dy) * PW + dx, [[PHW, src.shape[0]], [PW, nh], [1, W]]
    )


def build_body(nc, tc, T):
    x_in, out, fbuf = T["x_in"], T["out"], T["fbuf"]

    with tc.tile_pool(name="wpool", bufs=1) as wp:
        # ---- persistent weights ----
        w1s = wp.tile([128, 9 * 4 * HID], F32R)      # [ci%128, (tap, cc, co)]
        nc.sync.dma_start(w1s[:], T["w1t"][:])
        w2s = wp.tile([HID, 9 * HID], F32R)          # [ci, (tap, co)]
        nc.sync.dma_start(w2s[:], T["w2t"][:])
        w3s = wp.tile([HID, 9 * C], F32R)            # [ci, (tap, co)]
        nc.sync.dma_start(w3s[:], T["w3t"][:])
        dw1s = wp.tile([128, 4 * RED], F32R)         # [ci%128, (cc, co)]
        nc.sync.dma_start(dw1s[:], T["dw1t"][:])
        dw2s = wp.tile([RED, G * TPAD], F32R)
        nc.sync.dma_start(dw2s[:], T["dw2t"][:])
        b1s = wp.tile([HID, 1], F32)
        nc.sync.dma_start(b1s[:], T["b1"][:])
        b2s = wp.tile([HID, 1], F32)
        nc.sync.dma_start(b2s[:], T["b2"][:])
        b3s = wp.tile([128, 4], F32)
        nc.sync.dma_start(b3s[:], T["b3"][:])
        bnscs = wp.tile([RED, 1], F32)
        nc.sync.dma_start(bnscs[:], T["bnsc"][:])
        bnshs = wp.tile([RED, 1], F32)
        nc.sync.dma_start(bnshs[:], T["bnsh"][:])
        idbs = wp.tile([128, 128], BF16)
        nc.sync.dma_start(idbs[:], T["idb"][:])

        # ================= guide network + DCK =================
        with (
            tc.tile_pool(name="h12", bufs=1) as hp_,
            tc.tile_pool(name="cps", bufs=4, space="PSUM") as cps,
        ):
            h1 = hp_.tile([HID, PHW], F32R)
            nc.gpsimd.memset(h1[:].bitcast(F32), 0.0)
            h2 = hp_.tile([HID, PHW], F32R)
            nc.gpsimd.memset(h2[:].bitcast(F32), 0.0)

            with tc.tile_pool(name="xcp", bufs=1) as xcp:
                xc = []
                for cc in range(4):
                    t = xcp.tile([128, PHW], F32R, name=f"xc{cc}")
                    nc.gpsimd.memset(t[:].bitcast(F32), 0.0)
                    nc.sync.dma_start(
                        ap_of(t, PW + 1, [[PHW, 128], [PW, H], [1, W]]),
                        ap_of(T["xr"], cc * 128 * PIX, [[PIX, 128], [W, H], [1, W]]),
                    )
                    xc.append(t)

                # conv1: C->HID, 9 taps, 4 ci-chunks
                for oc in range(8):
                    ps = cps.tile([HID, 512], F32, tag="cv")
                    nmm = 0
                    for cc in range(4):
                        for tap in range(9):
                            nc.tensor.matmul(
                                ps[:],
                                w1s[:, (tap * 4 + cc) * HID:(tap * 4 + cc + 1) * HID],
                                conv_rhs(xc[cc], oc * 8, tap, 8),
                                start=(nmm == 0),
                                stop=(nmm == 35),
                            )
                            nmm += 1
                    nc.scalar.activation(
                        ap_of(h1, (oc * 8 + 1) * PW + 1, [[PHW, HID], [PW, 8], [1, W]]),
                        ps[:],
                        RELU,
                        bias=b1s[:],
                    )

            # conv2: HID->HID
            for oc in range(8):
                ps = cps.tile([HID, 512], F32, tag="cv")
                for tap in range(9):
                    nc.tensor.matmul(
                        ps[:],
                        w2s[:, tap * HID:(tap + 1) * HID],
                        conv_rhs(h1, oc * 8, tap, 8),
                        start=(tap == 0),
                        stop=(tap == 8),
                    )
                nc.scalar.activation(
                    ap_of(h2, (oc * 8 + 1) * PW + 1, [[PHW, HID], [PW, 8], [1, W]]),
                    ps[:],
                    RELU,
                    bias=b2s[:],
                )

            with tc.tile_pool(name="gd", bufs=1) as gp:
                guide = [gp.tile([128, PIX], F32R, name=f"gd{m}") for m in range(4)]
                # conv3: HID->C (4 m-chunks), output unpadded [128, 4096]
                for oc in range(8):
                    for mc in range(4):
                        ps = cps.tile([128, 512], F32, tag="cv")
                        for tap in range(9):
                            nc.tensor.matmul(
                                ps[:],
                                w3s[:, tap * C + mc * 128: tap * C + (mc + 1) * 128],
                                conv_rhs(h2, oc * 8, tap, 8),
                                start=(tap == 0),
                                stop=(tap == 8),
                            )
                        nc.scalar.activation(
                            guide[mc][:, oc * 512:(oc + 1) * 512],
                            ps[:],
                            RELU,
                            bias=b3s[:, mc:mc + 1],
                        )

                with tc.tile_pool(name="tfp", bufs=1) as tfp:
                    tf = tfp.tile([RED, PIX], F32R, name="tfeat")
                    for oc in range(8):
                        ps = cps.tile([RED, 512], F32, tag="cv")
                        for cc in range(4):
                            nc.tensor.matmul(
                                ps[:],
                                dw1s[:, cc * RED:(cc + 1) * RED],
                                guide[cc][:, oc * 512:(oc + 1) * 512],
                                start=(cc == 0),
                                stop=(cc == 3),
                            )
                        nc.scalar.activation(
                            tf[:, oc * 512:(oc + 1) * 512], ps[:], RELU,
                            bias=bnshs[:], scale=bnscs[:],
                        )

                    # dck2: RED -> G*TPAD (16 m-chunks of 128 = 2 groups x 64 taps)
                    with tc.tile_pool(name="fsp", bufs=3) as fsp:
                        for mch in range(16):
                            for oc in range(8):
                                ps = cps.tile([128, 512], F32, tag="cv")
                                nc.tensor.matmul(
                                    ps[:],
                                    dw2s[:, mch * 128:(mch + 1) * 128],
                                    tf[:, oc * 512:(oc + 1) * 512],
                                    start=True,
                                    stop=True,
                                )
                                fs = fsp.tile([128, 512], BF16, tag="fs")
                                nc.scalar.activation(fs[:], ps[:], COPY)
                                band, h0 = (oc * 8) // BH, (oc * 8) % BH
                                # fbuf[band, g, t, h, w]: partition (g_loc, t) -> 3-dim dram AP
                                nc.sync.dma_start(
                                    ap_of(
                                        fbuf,
                                        band * (G * TPAD * BH * W)
                                        + 2 * mch * (TPAD * BH * W)
                                        + h0 * W,
                                        [[TPAD * BH * W, 2], [BH * W, TPAD], [1, 512]],
                                    ),
                                    fs[:],
                                )

        # ================= xb build + apply =================
        with tc.tile_pool(name="xbp", bufs=1) as xbp:
            xb_e = xbp.tile([128, GC * CST], BF16)
            xb_o = xbp.tile([128, GC * CST], BF16)
            with tc.tile_pool(name="stg", bufs=1) as stp:
                for cc in range(4):
                    stage = stp.tile([128, 4 * CST], F32, tag="stage")
                    nc.gpsimd.memset(stage[:], 0.0)
                    for band in range(NB):
                        r_lo = max(0, band * BH - 3)
                        r_hi = min(H, band * BH + BH + 3)
                        rows = r_hi - r_lo
                        sro = r_lo - (band * BH - 3)  # stored-row offset
                        for cl in range(4):
                            nc.sync.dma_start(
                                ap_of(
                                    stage,
                                    band * G * (4 * CST) + cl * CST + sro * XW + 3,
                                    [[4 * CST, G], [XW, rows], [1, W]],
                                ),
                                ap_of(
                                    x_in,
                                    (cc * 4 + cl) * PIX + r_lo * W,
                                    [[GC * PIX, G], [W, rows], [1, W]],
                                ),
                            )
                    nc.vector.tensor_copy(
                        xb_e[:, cc * 4 * CST:(cc + 1) * 4 * CST], stage[:]
                    )
                    nc.vector.tensor_copy(
                        ap_of(xb_o, cc * 4 * CST, [[GC * CST, 128], [CST, 4], [XW, XH], [1, XW - 1]]),
                        ap_of(stage, 1, [[4 * CST, 128], [CST, 4], [XW, XH], [1, XW - 1]]),
                    )

            with (
                tc.tile_pool(name="app", bufs=2) as app,
                tc.tile_pool(name="ptp", bufs=3) as ptp,
                tc.tile_pool(name="osb", bufs=2) as op_,
                tc.tile_pool(name="aps", bufs=2, space="PSUM") as aps,
            ):
                for hp in range(8):
                    rt = app.tile([128, 2048], F32, tag="rt")
                    for band in range(NB):
                        nc.sync.dma_start(
                            rt[band * G:(band + 1) * G],
                            ap_of(
                                x_in,
                                band * BH * W + hp * 2 * W,
                                [[GC * PIX, G], [PIX, GC], [W, 2], [1, W]],
                            ),
                        )
                    pso = aps.tile([128, 2048], F32, tag="pso")
                    for tch in range(2):  # tap chunks: 0-31, 32-48
                        t0c, t1c = (0, 32) if tch == 0 else (32, NTAP)
                        ntc = t1c - t0c
                        # ft sbuf layout: (h2, t, w) h-major; fbuf is [band, g, t, h, w]
                        ft = app.tile([128, 2 * 32 * W], BF16, tag="ft")
                        for band in range(NB):
                            for r in range(2):
                                nc.sync.dma_start(
                                    ft[band * G:(band + 1) * G,
                                       r * ntc * W:(r + 1) * ntc * W],
                                    ap_of(
                                        fbuf,
                                        band * (G * TPAD * BH * W)
                                        + t0c * (BH * W)
                                        + (hp * 2 + r) * W,
                                        [[TPAD * BH * W, G], [BH * W, ntc], [1, W]],
                                    ),
                                )
                        for t in range(t0c, t1c):
                            dy, dx = t // K, t % K
                            if dx % 2 == 0:
                                xsrc, bc = xb_e, dx
                            else:
                                xsrc, bc = xb_o, dx - 1
                            in0 = ap_of(
                                xsrc,
                                (hp * 2 + dy) * XW + bc,
                                [[GC * CST, 128], [CST, GC], [XW, 2], [1, W]],
                            )
                            in1 = ap_of(
                                ft,
                                (t - t0c) * W,
                                [[2 * 32 * W, 128], [0, GC], [ntc * W, 2], [1, W]],
                            )
                            pt = ptp.tile([128, 2048], BF16, tag="pt")
                            pout = ap_of(pt, 0, [[2048, 128], [128, GC], [W, 2], [1, W]])
                            nc.vector.tensor_tensor(pout, in0, in1, op=MULT)
                            for j in range(4):
                                nc.tensor.matmul(
                                    pso[:, j * 512:(j + 1) * 512],
                                    idbs[:],
                                    pt[:, j * 512:(j + 1) * 512],
                                    start=(t == 0),
                                    stop=(t == NTAP - 1),
                                )
                    ob = op_.tile([128, 2048], F32, tag="ob")
                    nc.vector.scalar_tensor_tensor(
                        ob[:], pso[:], 1.0, rt[:],
                        op0=MULT, op1=mybir.AluOpType.add,
                    )
                    for band in range(NB):
                        nc.sync.dma_start(
                            ap_of(
                                out,
                                band * BH * W + hp * 2 * W,
                                [[GC * PIX, G], [PIX, GC], [W, 2], [1, W]],
                            ),
                            ob[band * G:(band + 1) * G],
                        )


def prep_weights(inputs):
    """Host-side weight transforms shared by all cores."""
    w1 = np.asarray(inputs["w1"], np.float32)   # [64, 512, 3, 3]
    w2 = np.asarray(inputs["w2"], np.float32)
    w3 = np.asarray(inputs["w3"], np.float32)   # [512, 64, 3, 3]
    dck_w1 = np.asarray(inputs["dck_w1"], np.float32)  # [128, 512, 1, 1]
    dck_w2 = np.asarray(inputs["dck_w2"], np.float32)  # [1568, 128, 1, 1]

    def tapify(w):  # [co, ci, 3, 3] -> [9, ci, co]
        return np.ascontiguousarray(w.transpose(2, 3, 1, 0).reshape(9, w.shape[1], w.shape[0]))

    w1sb = tapify(w1).reshape(9, 4, 128, HID).transpose(2, 0, 1, 3).reshape(128, 9 * 4 * HID)
    w2sb = tapify(w2).transpose(1, 0, 2).reshape(HID, 9 * HID)
    w3sb = tapify(w3).transpose(1, 0, 2).reshape(HID, 9 * C)
    dw1sb = dck_w1.reshape(RED, C).T.reshape(4, 128, RED).transpose(1, 0, 2).reshape(128, 4 * RED)

    bn_g = np.asarray(inputs["bn_gamma"], np.float32)
    bn_b = np.asarray(inputs["bn_beta"], np.float32)
    bn_m = np.asarray(inputs["bn_mean"], np.float32)
    bn_v = np.asarray(inputs["bn_var"], np.float32)
    inv_std = bn_g / np.sqrt(bn_v + 1e-5)
    shift = bn_b - bn_m * inv_std

    dw2 = dck_w2.reshape(G, NTAP, RED)          # [g, t, red]
    dw2p = np.zeros((G, TPAD, RED), np.float32)
    dw2p[:, :NTAP] = dw2
    dw2t = np.ascontiguousarray(dw2p.reshape(G * TPAD, RED).T)  # [red, g*64]

    return {
        "w1t": np.ascontiguousarray(w1sb),
        "b1": np.asarray(inputs["b1"], np.float32).reshape(HID, 1),
        "w2t": np.ascontiguousarray(w2sb),
        "b2": np.asarray(inputs["b2"], np.float32).reshape(HID, 1),
        "w3t": np.ascontiguousarray(w3sb),
        "b3": np.ascontiguousarray(np.asarray(inputs["b3"], np.float32).reshape(4, 128).T),
        "dw1t": np.ascontiguousarray(dw1sb),
        "bnsc": inv_std.reshape(RED, 1),
        "bnsh": shift.reshape(RED, 1),
        "dw2t": dw2t,
        "idb": np.eye(128).astype(ml_dtypes.bfloat16),
    }


_NC_CACHE = {}


def get_nc():
    if "nc" not in _NC_CACHE:
        _NC_CACHE["nc"] = build_nc()
    return _NC_CACHE["nc"]


def kernel(**inputs):
    nc = get_nc()
    wmap = prep_weights(inputs)
    x = np.asarray(inputs["x"], np.float32)
    in_maps = [
        {"x": np.ascontiguousarray(x[i]), "xr": np.ascontiguousarray(x[i]), **wmap}
        for i in range(N_CORES)
    ]
    res = bass_utils.run_bass_kernel_spmd(nc, in_maps, core_ids=list(range(N_CORES)))
    return np.stack([res.results[i]["out"] for i in range(N_CORES)]).astype(np.float32)



# revision 6
# speedup vs baseline: 1.2502x; 1.2502x over previous
"""Trainium2 Bass kernel for nn_DepthCue (dynamic-filter / CARAFE-style module).

Sharding: data-parallel over batch B=8 across the 8 NeuronCores (one sample
per core). Per core:
  - guide network (3x3 convs C->64->64->C) + DCK (1x1 convs + BN/ReLU) run on
    TensorE as shifted matmuls (f32r for 3x3 convs, bf16 for the 1x1 DCK
    convs); loops ordered tap-outer so the PE stream stays dense.
  - dynamic-filter apply: partitions = (h-band, group); per-tap elementwise
    multiply on VectorE in bf16 (filters broadcast across the 16 group
    channels via a step-0 AP dim); tap accumulation via identity-matmul into
    PSUM (fp32), two taps per matmul via a step-0 output AP dim.
  - host prepares padded/shifted input layouts (xcr for the conv, xpe/xpo for
    the apply, both halos baked in) and adds the residual x at the end.
"""

import numpy as np
import ml_dtypes

import concourse.bass as bass
import concourse.bacc as bacc
import concourse.mybir as mybir
from concourse import bass_utils
from concourse.tile import TileContext

F32 = mybir.dt.float32
F32R = mybir.dt.float32r
BF16 = mybir.dt.bfloat16
MULT = mybir.AluOpType.mult
RELU = mybir.ActivationFunctionType.Relu
COPY = mybir.ActivationFunctionType.Copy

N_CORES = 8
C, H, W = 512, 64, 64
HID = 64          # guide-net hidden channels
RED = 128         # DCK reduction channels
G = 32            # groups
GC = 16           # channels per group
K = 7             # dynamic kernel size
NTAP = K * K      # 49
TPAD = 64         # taps padded to 64 in dck2 weights / fbuf
NB = 4            # h-bands (partition index = band*32 + g)
BH = 16           # rows per band
XH = BH + 6       # xb stored rows per band (halo 3 top + 3 bottom)
XW = W + 6        # xb stored cols (halo 3 + 3)
CST = XH * XW     # per-channel stride in xb free dim (1540)
PIX = H * W       # 4096
PW = W + 2        # padded width for conv intermediates (66)
PHW = (H + 2) * PW
XCH = 34 * PW     # xc half-pass stored elems per partition (rows 0..34 / 32..66)


def ap_of(t, offset, dims):
    """Raw AP over tile/dram tensor t: dims = [[step, count], ...] (dim0 = partition for sbuf)."""
    base = t if isinstance(t, bass.AP) else t[:]
    return bass.AP(tensor=base.tensor, offset=offset, ap=[list(d) for d in dims])


def build_nc():
    nc = bacc.Bacc(trn_type="TRN2", target_bir_lowering=False, debug=False)

    T = {}
    T["xcr"] = nc.dram_tensor("xcr", [C, PHW], F32R, kind="ExternalInput").ap()
    T["xpe"] = nc.dram_tensor("xpe", [128, GC * CST], BF16, kind="ExternalInput").ap()
    T["xpo"] = nc.dram_tensor("xpo", [128, GC * CST], BF16, kind="ExternalInput").ap()
    T["w1t"] = nc.dram_tensor("w1t", [128, 9 * 4 * HID], F32R, kind="ExternalInput").ap()
    T["b1"] = nc.dram_tensor("b1", [HID, 1], F32, kind="ExternalInput").ap()
    T["w2t"] = nc.dram_tensor("w2t", [HID, 9 * HID], F32R, kind="ExternalInput").ap()
    T["b2"] = nc.dram_tensor("b2", [HID, 1], F32, kind="ExternalInput").ap()
    T["w3t"] = nc.dram_tensor("w3t", [HID, 9 * C], F32R, kind="ExternalInput").ap()
    T["b3"] = nc.dram_tensor("b3", [128, 4], F32, kind="ExternalInput").ap()
    T["dw1t"] = nc.dram_tensor("dw1t", [128, 4 * RED], BF16, kind="ExternalInput").ap()
    T["bnsc"] = nc.dram_tensor("bnsc", [RED, 1], F32, kind="ExternalInput").ap()
    T["bnsh"] = nc.dram_tensor("bnsh", [RED, 1], F32, kind="ExternalInput").ap()
    T["dw2t"] = nc.dram_tensor("dw2t", [RED, G * TPAD], BF16, kind="ExternalInput").ap()
    T["idb"] = nc.dram_tensor("idb", [128, 128], BF16, kind="ExternalInput").ap()
    T["out"] = nc.dram_tensor("out", [128, 8 * 2048], F32, kind="ExternalOutput").ap()
    # filters scratch: [band, g, tap(64), h(16), w] bf16
    T["fbuf"] = nc.dram_tensor("fbuf", [NB, G, TPAD, BH, W], BF16, kind="Internal").ap()

    with TileContext(nc) as tc:
        build_body(nc, tc, T)
    nc.compile()
    return nc


def conv_rhs(src, pstride, r0, tap, nh=8):
    dy, dx = tap // 3, tap % 3
    return ap_of(
        src, (r0 + dy) * PW + dx, [[pstride, src.shape[0]], [PW, nh], [1, W]]
    )


def build_body(nc, tc, T):
    with tc.tile_pool(name="wper", bufs=1) as wper:
        # persistent across conv + apply
        idbs = wper.tile([128, 128], BF16)
        nc.sync.dma_start(idbs[:], T["idb"][:])
        xpe = wper.tile([128, GC * CST], BF16)
        xpo = wper.tile([128, GC * CST], BF16)
        # big apply inputs stream in on the scalar ring while conv runs
        nc.scalar.dma_start(xpe[:], T["xpe"][:])
        nc.scalar.dma_start(xpo[:], T["xpo"][:])

        with tc.tile_pool(name="wcv", bufs=1) as wcv:
            w2s = wcv.tile([HID, 9 * HID], F32R)          # [ci, (tap, co)]
            nc.sync.dma_start(w2s[:], T["w2t"][:])
            w3s = wcv.tile([HID, 9 * C], F32R)            # [ci, (tap, co)]
            nc.sync.dma_start(w3s[:], T["w3t"][:])
            dw1s = wcv.tile([128, 4 * RED], BF16)         # [ci%128, (cc, co)]
            nc.sync.dma_start(dw1s[:], T["dw1t"][:])
            dw2s = wcv.tile([RED, G * TPAD], BF16)
            nc.sync.dma_start(dw2s[:], T["dw2t"][:])
            b2s = wcv.tile([HID, 1], F32)
            nc.sync.dma_start(b2s[:], T["b2"][:])
            b3s = wcv.tile([128, 4], F32)
            nc.sync.dma_start(b3s[:], T["b3"][:])
            bnscs = wcv.tile([RED, 1], F32)
            nc.sync.dma_start(bnscs[:], T["bnsc"][:])
            bnshs = wcv.tile([RED, 1], F32)
            nc.sync.dma_start(bnshs[:], T["bnsh"][:])

            conv_net(nc, tc, T, w2s, w3s, dw1s, dw2s,
                     b2s, b3s, bnscs, bnshs)

        apply_filters(nc, tc, T, xpe, xpo, idbs)


def conv_net(nc, tc, T, w2s, w3s, dw1s, dw2s, b2s, b3s, bnscs, bnshs):
    fbuf = T["fbuf"]
    with (
        tc.tile_pool(name="hp2", bufs=1) as hp2,
        tc.tile_pool(name="cps", bufs=8, space="PSUM") as cps,
    ):
        h1 = hp2.tile([HID, PHW], F32R)
        nc.gpsimd.memset(h1[:].bitcast(F32), 0.0)
        h2 = hp2.tile([HID, PHW], F32R)
        nc.gpsimd.memset(h2[:].bitcast(F32), 0.0)

        # ---- conv1: C->HID, two row-halves (xc half-pass tiles) ----
        with tc.tile_pool(name="wc1", bufs=1) as wc1:
            w1s = wc1.tile([128, 9 * 4 * HID], F32R)      # [ci%128, (tap, cc, co)]
            nc.sync.dma_start(w1s[:], T["w1t"][:])
            b1s = wc1.tile([HID, 1], F32)
            nc.sync.dma_start(b1s[:], T["b1"][:])
            with tc.tile_pool(name="xcp", bufs=4) as xcp:
                xc = []
                for cc in range(4):
                    t = xcp.tile([128, XCH], F32R, tag="xc", name=f"xc_{cc}")
                    nc.sync.dma_start(
                        t[:], ap_of(T["xcr"], cc * 128 * PHW, [[PHW, 128], [1, XCH]])
                    )
                    xc.append(t)
                for half in range(2):
                    ps1 = [cps.tile([HID, 512], F32, tag="cv", name=f"ps1_{half}_{i}") for i in range(4)]
                    for cc in range(4):
                        for tap in range(9):
                            for o in range(4):
                                nc.tensor.matmul(
                                    ps1[o],
                                    w1s[:, (tap * 4 + cc) * HID:(tap * 4 + cc + 1) * HID],
                                    conv_rhs(xc[cc], XCH, o * 8, tap),
                                    start=(cc == 0 and tap == 0),
                                    stop=(cc == 3 and tap == 8),
                                )
                        if half == 0:
                            # second-half rows of this ci chunk, reusing the slot
                            t = xcp.tile([128, XCH], F32R, tag="xc", name=f"xcb_{cc}")
                            nc.sync.dma_start(
                                t[:],
                                ap_of(T["xcr"], cc * 128 * PHW + 32 * PW,
                                      [[PHW, 128], [1, XCH]]),
                            )
                            xc[cc] = t
                    for o in range(4):
                        oc = half * 4 + o
                        nc.scalar.activation(
                            ap_of(h1, (oc * 8 + 1) * PW + 1, [[PHW, HID], [PW, 8], [1, W]]),
                            ps1[o][:],
                            RELU,
                            bias=b1s[:],
                        )

        # ---- conv2: HID->HID, tap-outer ----
        ps2 = [cps.tile([HID, 512], F32, tag="cv", name=f"ps2_{i}") for i in range(8)]
        for tap in range(9):
            for oc in range(8):
                nc.tensor.matmul(
                    ps2[oc],
                    w2s[:, tap * HID:(tap + 1) * HID],
                    conv_rhs(h1, PHW, oc * 8, tap),
                    start=(tap == 0),
                    stop=(tap == 8),
                )
        for oc in range(8):
            nc.scalar.activation(
                ap_of(h2, (oc * 8 + 1) * PW + 1, [[PHW, HID], [PW, 8], [1, W]]),
                ps2[oc][:],
                RELU,
                bias=b2s[:],
            )

        with tc.tile_pool(name="gdp", bufs=1) as gdp:
            guide = [gdp.tile([128, PIX], BF16, name=f"gd{m}") for m in range(4)]
            # ---- conv3: HID->C (4 m-chunks), tap-outer within each chunk ----
            for mc in range(4):
                ps3 = [cps.tile([128, 512], F32, tag="cv", name=f"ps3_{mc}_{i}") for i in range(8)]
                for tap in range(9):
                    for oc in range(8):
                        nc.tensor.matmul(
                            ps3[oc],
                            w3s[:, tap * C + mc * 128: tap * C + (mc + 1) * 128],
                            conv_rhs(h2, PHW, oc * 8, tap),
                            start=(tap == 0),
                            stop=(tap == 8),
                        )
                for oc in range(8):
                    nc.scalar.activation(
                        guide[mc][:, oc * 512:(oc + 1) * 512],
                        ps3[oc][:],
                        RELU,
                        bias=b3s[:, mc:mc + 1],
                    )

            # ---- dck1: C->RED 1x1 + BN + ReLU (cc-outer) ----
            with tc.tile_pool(name="tfp", bufs=1) as tfp:
                tf = tfp.tile([RED, PIX], BF16, name="tfeat")
                psd = [cps.tile([RED, 512], F32, tag="cv", name=f"psd_{i}") for i in range(8)]
                for cc in range(4):
                    for oc in range(8):
                        nc.tensor.matmul(
                            psd[oc],
                            dw1s[:, cc * RED:(cc + 1) * RED],
                            guide[cc][:, oc * 512:(oc + 1) * 512],
                            start=(cc == 0),
                            stop=(cc == 3),
                        )
                for oc in range(8):
                    nc.scalar.activation(
                        tf[:, oc * 512:(oc + 1) * 512], psd[oc][:], RELU,
                        bias=bnshs[:], scale=bnscs[:],
                    )

                # ---- dck2: RED -> G*TPAD; [band,g,h,tap,w] fbuf layout ----
                with tc.tile_pool(name="fsp", bufs=4) as fsp:
                    for mch in range(16):
                        for oc in range(8):
                            ps = cps.tile([128, 512], F32, tag="cv", name=f"fps_{mch}_{oc}")
                            nc.tensor.matmul(
                                ps[:],
                                dw2s[:, mch * 128:(mch + 1) * 128],
                                tf[:, oc * 512:(oc + 1) * 512],
                                start=True,
                                stop=True,
                            )
                            fs = fsp.tile([128, 512], BF16, tag="fs", name=f"fs_{mch}_{oc}")
                            if oc % 2 == 0:
                                nc.scalar.activation(fs[:], ps[:], COPY)
                            else:
                                nc.vector.tensor_copy(fs[:], ps[:])
                            band, h0 = (oc * 8) // BH, (oc * 8) % BH
                            # psum [128=(gl,t), 512=(8h,64w)] -> fbuf[band, 2mch+gl, t, h0+h, w]
                            nc.sync.dma_start(
                                ap_of(
                                    fbuf,
                                    band * (G * TPAD * BH * W)
                                    + 2 * mch * (TPAD * BH * W)
                                    + h0 * W,
                                    [[TPAD * BH * W, 2], [BH * W, TPAD], [1, 512]],
                                ),
                                fs[:],
                            )


def apply_filters(nc, tc, T, xpe, xpo, idbs):
    fbuf, out = T["fbuf"], T["out"]
    with (
        tc.tile_pool(name="ftp", bufs=3) as ftp,
        tc.tile_pool(name="ptp", bufs=6) as ptp,
        tc.tile_pool(name="osb", bufs=2) as osb,
        tc.tile_pool(name="aps", bufs=2, space="PSUM") as aps,
    ):
        for hp in range(8):
            pso = aps.tile([128, 2048], F32, tag="pso")
            # filter chunks: taps [0,32), [32,49); sbuf layout (row, tap, w)
            fts = []
            chunks = [(0, 32), (32, NTAP)]
            for t0, t1 in chunks:
                ntc = t1 - t0
                ft = ftp.tile([128, 2 * 32 * W], BF16, tag="ft", name=f"ft_{hp}_{t0}")
                nc.sync.dma_start(
                    ap_of(ft, 0, [[2 * 32 * W, 128], [1, ntc * 2 * W]]),
                    ap_of(
                        fbuf,
                        t0 * (BH * W) + (hp * 2) * W,
                        [[TPAD * BH * W, 128], [BH * W, ntc], [1, 2 * W]],
                    ),
                )
                fts.append(ft)

            def tt_product(t, dst, dst_off):
                """pt[dst_off half] = x_shift(tap t) * filter(tap t)."""
                ci = 0 if t < 32 else 1
                t0, t1 = chunks[ci]
                dy, dx = t // K, t % K
                xsrc, bc = (xpe, dx) if dx % 2 == 0 else (xpo, dx - 1)
                in0 = ap_of(
                    xsrc, (hp * 2 + dy) * XW + bc,
                    [[GC * CST, 128], [CST, GC], [XW, 2], [1, W]],
                )
                in1 = ap_of(
                    fts[ci], (t - t0) * 2 * W,
                    [[2 * 32 * W, 128], [0, GC], [W, 2], [1, W]],
                )
                pout = ap_of(dst, dst_off, [[2048, 128], [128, GC], [W, 2], [1, W]])
                nc.vector.tensor_tensor(pout, in0, in1, op=MULT)

            # one TT product + 4 accumulate matmuls per tap
            for t in range(NTAP):
                pt = ptp.tile([128, 2048], BF16, tag="pt", name=f"pt_{hp}_{t}")
                tt_product(t, pt, 0)
                for j in range(4):
                    nc.tensor.matmul(
                        pso[:, j * 512:(j + 1) * 512],
                        idbs[:],
                        ap_of(pt, j * 512, [[2048, 128], [1, 512]]),
                        start=(t == 0),
                        stop=(t == NTAP - 1),
                    )
            ob = osb.tile([128, 2048], F32, tag="ob")
            nc.scalar.activation(ob[:], pso[:], COPY)
            nc.sync.dma_start(
                ap_of(out, hp * 2048, [[8 * 2048, 128], [1, 2048]]), ob[:]
            )


def prep_weights(inputs):
    """Host-side weight transforms shared by all cores."""
    w1 = np.asarray(inputs["w1"], np.float32)   # [64, 512, 3, 3]
    w2 = np.asarray(inputs["w2"], np.float32)
    w3 = np.asarray(inputs["w3"], np.float32)   # [512, 64, 3, 3]
    dck_w1 = np.asarray(inputs["dck_w1"], np.float32)  # [128, 512, 1, 1]
    dck_w2 = np.asarray(inputs["dck_w2"], np.float32)  # [1568, 128, 1, 1]

    def tapify(w):  # [co, ci, 3, 3] -> [9, ci, co]
        return np.ascontiguousarray(w.transpose(2, 3, 1, 0).reshape(9, w.shape[1], w.shape[0]))

    w1sb = tapify(w1).reshape(9, 4, 128, HID).transpose(2, 0, 1, 3).reshape(128, 9 * 4 * HID)
    w2sb = tapify(w2).transpose(1, 0, 2).reshape(HID, 9 * HID)
    w3sb = tapify(w3).transpose(1, 0, 2).reshape(HID, 9 * C)
    dw1sb = dck_w1.reshape(RED, C).T.reshape(4, 128, RED).transpose(1, 0, 2).reshape(128, 4 * RED)

    bn_g = np.asarray(inputs["bn_gamma"], np.float32)
    bn_b = np.asarray(inputs["bn_beta"], np.float32)
    bn_m = np.asarray(inputs["bn_mean"], np.float32)
    bn_v = np.asarray(inputs["bn_var"], np.float32)
    inv_std = bn_g / np.sqrt(bn_v + 1e-5)
    shift = bn_b - bn_m * inv_std

    dw2 = dck_w2.reshape(G, NTAP, RED)          # [g, t, red]
    dw2p = np.zeros((G, TPAD, RED), np.float32)
    dw2p[:, :NTAP] = dw2
    dw2t = np.ascontiguousarray(dw2p.reshape(G * TPAD, RED).T)  # [red, g*64]

    return {
        "w1t": np.ascontiguousarray(w1sb),
        "b1": np.asarray(inputs["b1"], np.float32).reshape(HID, 1),
        "w2t": np.ascontiguousarray(w2sb),
        "b2": np.asarray(inputs["b2"], np.float32).reshape(HID, 1),
        "w3t": np.ascontiguousarray(w3sb),
        "b3": np.ascontiguousarray(np.asarray(inputs["b3"], np.float32).reshape(4, 128).T),
        "dw1t": dw1sb.astype(ml_dtypes.bfloat16),
        "bnsc": inv_std.reshape(RED, 1),
        "bnsh": shift.reshape(RED, 1),
        "dw2t": dw2t.astype(ml_dtypes.bfloat16),
        "idb": np.eye(128).astype(ml_dtypes.bfloat16),
    }


def prep_x(xi):
    """Per-sample input layouts: xcr (conv, pad 1), xpe/xpo (apply, pad 3 + shift)."""
    x = np.asarray(xi, np.float32)
    xcr = np.pad(x, ((0, 0), (1, 1), (1, 1))).reshape(C, PHW)
    xp3 = np.pad(x, ((0, 0), (3, 3), (3, 3))).astype(ml_dtypes.bfloat16)  # [512,70,70]
    xps = np.zeros_like(xp3)
    xps[:, :, :-1] = xp3[:, :, 1:]
    def bands(a):  # [512,70,70] -> [128=(band,g), GC*CST]
        v = a.reshape(G, GC, H + 6, XW)
        o = np.empty((NB, G, GC, XH, XW), ml_dtypes.bfloat16)
        for b in range(NB):
            o[b] = v[:, :, b * BH:b * BH + XH]
        return o.reshape(128, GC * CST)
    return {
        "xcr": np.ascontiguousarray(xcr),
        "xpe": np.ascontiguousarray(bands(xp3)),
        "xpo": np.ascontiguousarray(bands(xps)),
    }


def unpack_out(raw, x):
    """[128, 8*2048] filter-term -> [C,H,W], plus residual x."""
    r = np.asarray(raw, np.float32).reshape(NB, G, 8, GC, 2, W)
    r = r.transpose(1, 3, 0, 2, 4, 5).reshape(C, H, W)
    return r + np.asarray(x, np.float32)


_NC_CACHE = {}


def get_nc():
    if "nc" not in _NC_CACHE:
        _NC_CACHE["nc"] = build_nc()
    return _NC_CACHE["nc"]


def make_in_maps(inputs):
    wmap = prep_weights(inputs)
    x = np.asarray(inputs["x"], np.float32)
    return [{**prep_x(x[i]), **wmap} for i in range(N_CORES)]


def kernel(**inputs):
    nc = get_nc()
    x = np.asarray(inputs["x"], np.float32)
    in_maps = make_in_maps(inputs)
    res = bass_utils.run_bass_kernel_spmd(nc, in_maps, core_ids=list(range(N_CORES)))
    return np.stack(
        [unpack_out(res.results[i]["out"], x[i]) for i in range(N_CORES)]
    ).astype(np.float32)


# revision 7
# speedup vs baseline: 1.3225x; 1.0578x over previous
"""Trainium2 Bass kernel for nn_DepthCue (dynamic-filter / CARAFE-style module).

Sharding: data-parallel over batch B=8 across the 8 NeuronCores (one sample
per core). Per core:
  - guide network (3x3 convs C->64->64->C) + DCK (1x1 convs + BN/ReLU) run on
    TensorE as shifted matmuls (f32r for 3x3 convs, bf16 for the 1x1 DCK
    convs); loops ordered tap-outer so the PE stream stays dense.
  - dynamic-filter apply: partitions = (h-band, group); per-tap elementwise
    multiply on VectorE in bf16 (filters broadcast across the 16 group
    channels via a step-0 AP dim); tap accumulation via identity-matmul into
    PSUM (fp32), two taps per matmul via a step-0 output AP dim.
  - host prepares padded/shifted input layouts (xcr for the conv, xpe/xpo for
    the apply, both halos baked in) and adds the residual x at the end.
"""

import numpy as np
import ml_dtypes

import concourse.bass as bass
import concourse.bacc as bacc
import concourse.mybir as mybir
from concourse import bass_utils
from concourse.tile import TileContext

F32 = mybir.dt.float32
F32R = mybir.dt.float32r
BF16 = mybir.dt.bfloat16
MULT = mybir.AluOpType.mult
RELU = mybir.ActivationFunctionType.Relu
COPY = mybir.ActivationFunctionType.Copy

N_CORES = 8
C, H, W = 512, 64, 64
HID = 64          # guide-net hidden channels
RED = 128         # DCK reduction channels
G = 32            # groups
GC = 16           # channels per group
K = 7             # dynamic kernel size
NTAP = K * K      # 49
TPAD = 64         # taps padded to 64 in dck2 weights / fbuf
NB = 4            # h-bands (partition index = band*32 + g)
BH = 16           # rows per band
XH = BH + 6       # xb stored rows per band (halo 3 top + 3 bottom)
XW = W + 6        # xb stored cols (halo 3 + 3)
CST = XH * XW     # per-channel stride in xb free dim (1540)
PIX = H * W       # 4096
PW = W + 2        # padded width for conv intermediates (66)
PHW = (H + 2) * PW
XCH = 34 * PW     # xc half-pass stored elems per partition (rows 0..34 / 32..66)


def ap_of(t, offset, dims):
    """Raw AP over tile/dram tensor t: dims = [[step, count], ...] (dim0 = partition for sbuf)."""
    base = t if isinstance(t, bass.AP) else t[:]
    return bass.AP(tensor=base.tensor, offset=offset, ap=[list(d) for d in dims])


def build_nc():
    nc = bacc.Bacc(trn_type="TRN2", target_bir_lowering=False, debug=False)

    T = {}
    T["xcr"] = nc.dram_tensor("xcr", [C, PHW], F32R, kind="ExternalInput").ap()
    T["xpe"] = nc.dram_tensor("xpe", [128, GC * CST], BF16, kind="ExternalInput").ap()
    T["xpo"] = nc.dram_tensor("xpo", [128, GC * CST], BF16, kind="ExternalInput").ap()
    T["w1t"] = nc.dram_tensor("w1t", [128, 9 * 4 * HID], F32R, kind="ExternalInput").ap()
    T["b1"] = nc.dram_tensor("b1", [HID, 1], F32, kind="ExternalInput").ap()
    T["w2t"] = nc.dram_tensor("w2t", [HID, 9 * HID], F32R, kind="ExternalInput").ap()
    T["b2"] = nc.dram_tensor("b2", [HID, 1], F32, kind="ExternalInput").ap()
    T["w3t"] = nc.dram_tensor("w3t", [HID, 9 * C], F32R, kind="ExternalInput").ap()
    T["b3"] = nc.dram_tensor("b3", [128, 4], F32, kind="ExternalInput").ap()
    T["dw1t"] = nc.dram_tensor("dw1t", [128, 4 * RED], BF16, kind="ExternalInput").ap()
    T["bnsc"] = nc.dram_tensor("bnsc", [RED, 1], F32, kind="ExternalInput").ap()
    T["bnsh"] = nc.dram_tensor("bnsh", [RED, 1], F32, kind="ExternalInput").ap()
    T["dw2t"] = nc.dram_tensor("dw2t", [RED, G * TPAD], BF16, kind="ExternalInput").ap()
    T["idb"] = nc.dram_tensor("idb", [128, 128], BF16, kind="ExternalInput").ap()
    T["out"] = nc.dram_tensor("out", [128, 8 * 2048], F32, kind="ExternalOutput").ap()
    # filters scratch: [band, g, tap(64), h(16), w] bf16
    T["fbuf"] = nc.dram_tensor("fbuf", [NB, G, TPAD, BH, W], BF16, kind="Internal").ap()

    with TileContext(nc) as tc:
        build_body(nc, tc, T)
    nc.compile()
    return nc


def conv_rhs(src, pstride, r0, tap, nh=8):
    dy, dx = tap // 3, tap % 3
    return ap_of(
        src, (r0 + dy) * PW + dx, [[pstride, src.shape[0]], [PW, nh], [1, W]]
    )


def build_body(nc, tc, T):
    with tc.tile_pool(name="wper", bufs=1) as wper:
        # persistent across conv + apply
        idbs = wper.tile([128, 128], BF16)
        nc.sync.dma_start(idbs[:], T["idb"][:])
        xpe = wper.tile([128, GC * CST], BF16)
        xpo = wper.tile([128, GC * CST], BF16)

        with tc.tile_pool(name="wcv", bufs=1) as wcv:
            w2s = wcv.tile([HID, 9 * HID], F32R)          # [ci, (tap, co)]
            w3s = wcv.tile([HID, 9 * C], F32R)            # [ci, (tap, co)]
            dw1s = wcv.tile([128, 4 * RED], BF16)         # [ci%128, (cc, co)]
            dw2s = wcv.tile([RED, G * TPAD], BF16)
            b2s = wcv.tile([HID, 1], F32)
            b3s = wcv.tile([128, 4], F32)
            bnscs = wcv.tile([RED, 1], F32)
            bnshs = wcv.tile([RED, 1], F32)
            # loads issued inside conv_net after conv1 is queued, so conv1's
            # own inputs go first on the ring
            deferred = [
                (w2s, "w2t"), (w3s, "w3t"), (dw1s, "dw1t"), (dw2s, "dw2t"),
                (b2s, "b2"), (b3s, "b3"), (bnscs, "bnsc"), (bnshs, "bnsh"),
            ]
            conv_net(nc, tc, T, w2s, w3s, dw1s, dw2s,
                     b2s, b3s, bnscs, bnshs, deferred, xpe, xpo)

        apply_filters(nc, tc, T, xpe, xpo, idbs)


def conv_net(nc, tc, T, w2s, w3s, dw1s, dw2s, b2s, b3s, bnscs, bnshs, deferred, xpe, xpo):
    fbuf = T["fbuf"]
    with (
        tc.tile_pool(name="hp2", bufs=1) as hp2,
        tc.tile_pool(name="cps", bufs=8, space="PSUM") as cps,
    ):
        h1 = hp2.tile([HID, PHW], F32R)
        nc.gpsimd.memset(h1[:].bitcast(F32), 0.0)
        h2 = hp2.tile([HID, PHW], F32R)
        nc.gpsimd.memset(h2[:].bitcast(F32), 0.0)

        # ---- conv1: C->HID, two row-halves (xc half-pass tiles) ----
        with tc.tile_pool(name="wc1", bufs=1) as wc1:
            w1s = wc1.tile([128, 9 * 4 * HID], F32R)      # [ci%128, (tap, cc, co)]
            nc.sync.dma_start(w1s[:], T["w1t"][:])
            b1s = wc1.tile([HID, 1], F32)
            nc.sync.dma_start(b1s[:], T["b1"][:])
            with tc.tile_pool(name="xcp", bufs=4) as xcp:
                xc = []
                for cc in range(4):
                    t = xcp.tile([128, XCH], F32R, tag="xc", name=f"xc_{cc}")
                    nc.sync.dma_start(
                        t[:], ap_of(T["xcr"], cc * 128 * PHW, [[PHW, 128], [1, XCH]])
                    )
                    xc.append(t)
                for half in range(2):
                    ps1 = [cps.tile([HID, 512], F32, tag="cv", name=f"ps1_{half}_{i}") for i in range(4)]
                    for cc in range(4):
                        for tap in range(9):
                            for o in range(4):
                                nc.tensor.matmul(
                                    ps1[o],
                                    w1s[:, (tap * 4 + cc) * HID:(tap * 4 + cc + 1) * HID],
                                    conv_rhs(xc[cc], XCH, o * 8, tap),
                                    start=(cc == 0 and tap == 0),
                                    stop=(cc == 3 and tap == 8),
                                )
                        if half == 0:
                            # second-half rows of this ci chunk, reusing the slot
                            t = xcp.tile([128, XCH], F32R, tag="xc", name=f"xcb_{cc}")
                            nc.sync.dma_start(
                                t[:],
                                ap_of(T["xcr"], cc * 128 * PHW + 32 * PW,
                                      [[PHW, 128], [1, XCH]]),
                            )
                            xc[cc] = t
                    for o in range(4):
                        oc = half * 4 + o
                        nc.scalar.activation(
                            ap_of(h1, (oc * 8 + 1) * PW + 1, [[PHW, HID], [PW, 8], [1, W]]),
                            ps1[o][:],
                            RELU,
                            bias=b1s[:],
                        )

        for tile_, key in deferred:
            nc.sync.dma_start(tile_[:], T[key][:])
        # big apply inputs stream in on the scalar ring during conv2/conv3
        nc.scalar.dma_start(xpe[:], T["xpe"][:])
        nc.scalar.dma_start(xpo[:], T["xpo"][:])

        # ---- conv2: HID->HID, tap-outer ----
        ps2 = [cps.tile([HID, 512], F32, tag="cv", name=f"ps2_{i}") for i in range(8)]
        for tap in range(9):
            for oc in range(8):
                nc.tensor.matmul(
                    ps2[oc],
                    w2s[:, tap * HID:(tap + 1) * HID],
                    conv_rhs(h1, PHW, oc * 8, tap),
                    start=(tap == 0),
                    stop=(tap == 8),
                )
        for oc in range(8):
            nc.scalar.activation(
                ap_of(h2, (oc * 8 + 1) * PW + 1, [[PHW, HID], [PW, 8], [1, W]]),
                ps2[oc][:],
                RELU,
                bias=b2s[:],
            )

        with tc.tile_pool(name="gdp", bufs=1) as gdp:
            guide = [gdp.tile([128, PIX], BF16, name=f"gd{m}") for m in range(4)]
            # ---- conv3: HID->C (4 m-chunks), tap-outer within each chunk ----
            for mc in range(4):
                ps3 = [cps.tile([128, 512], F32, tag="cv", name=f"ps3_{mc}_{i}") for i in range(8)]
                for tap in range(9):
                    for oc in range(8):
                        nc.tensor.matmul(
                            ps3[oc],
                            w3s[:, tap * C + mc * 128: tap * C + (mc + 1) * 128],
                            conv_rhs(h2, PHW, oc * 8, tap),
                            start=(tap == 0),
                            stop=(tap == 8),
                        )
                for oc in range(8):
                    nc.scalar.activation(
                        guide[mc][:, oc * 512:(oc + 1) * 512],
                        ps3[oc][:],
                        RELU,
                        bias=b3s[:, mc:mc + 1],
                    )

            # ---- dck1: C->RED 1x1 + BN + ReLU (cc-outer) ----
            with tc.tile_pool(name="tfp", bufs=1) as tfp:
                tf = tfp.tile([RED, PIX], BF16, name="tfeat")
                psd = [cps.tile([RED, 512], F32, tag="cv", name=f"psd_{i}") for i in range(8)]
                for cc in range(4):
                    for oc in range(8):
                        nc.tensor.matmul(
                            psd[oc],
                            dw1s[:, cc * RED:(cc + 1) * RED],
                            guide[cc][:, oc * 512:(oc + 1) * 512],
                            start=(cc == 0),
                            stop=(cc == 3),
                        )
                for oc in range(8):
                    nc.scalar.activation(
                        tf[:, oc * 512:(oc + 1) * 512], psd[oc][:], RELU,
                        bias=bnshs[:], scale=bnscs[:],
                    )

                # ---- dck2: RED -> G*TPAD; [band,g,tap,h,w] fbuf layout ----
                with tc.tile_pool(name="fsp", bufs=4) as fsp:
                    for mch in range(16):
                        for ocp in range(4):
                            fs = fsp.tile([128, 1024], BF16, tag="fs", name=f"fs_{mch}_{ocp}")
                            for sub in range(2):
                                oc = 2 * ocp + sub
                                ps = cps.tile([128, 512], F32, tag="cv", name=f"fps_{mch}_{oc}")
                                nc.tensor.matmul(
                                    ps[:],
                                    dw2s[:, mch * 128:(mch + 1) * 128],
                                    tf[:, oc * 512:(oc + 1) * 512],
                                    start=True,
                                    stop=True,
                                )
                                if sub == 0:
                                    nc.scalar.activation(fs[:, :512], ps[:], COPY)
                                else:
                                    nc.vector.tensor_copy(fs[:, 512:], ps[:])
                            # rows 0..16 of band=ocp, 2 groups, all 64 taps
                            eng = nc.sync if (mch + ocp) % 2 == 0 else nc.scalar
                            eng.dma_start(
                                ap_of(
                                    fbuf,
                                    ocp * (G * TPAD * BH * W)
                                    + 2 * mch * (TPAD * BH * W),
                                    [[TPAD * BH * W, 2], [BH * W, TPAD], [1, 1024]],
                                ),
                                fs[:],
                            )


def apply_filters(nc, tc, T, xpe, xpo, idbs):
    fbuf, out = T["fbuf"], T["out"]
    with (
        tc.tile_pool(name="ftp", bufs=3) as ftp,
        tc.tile_pool(name="ptp", bufs=6) as ptp,
        tc.tile_pool(name="osb", bufs=2) as osb,
        tc.tile_pool(name="aps", bufs=2, space="PSUM") as aps,
    ):
        for hp in range(8):
            pso = aps.tile([128, 2048], F32, tag="pso")
            # filter chunks: taps [0,32), [32,49); sbuf layout (row, tap, w)
            fts = []
            chunks = [(0, 32), (32, NTAP)]
            for t0, t1 in chunks:
                ntc = t1 - t0
                ft = ftp.tile([128, 2 * 32 * W], BF16, tag="ft", name=f"ft_{hp}_{t0}")
                nc.sync.dma_start(
                    ap_of(ft, 0, [[2 * 32 * W, 128], [1, ntc * 2 * W]]),
                    ap_of(
                        fbuf,
                        t0 * (BH * W) + (hp * 2) * W,
                        [[TPAD * BH * W, 128], [BH * W, ntc], [1, 2 * W]],
                    ),
                )
                fts.append(ft)

            def tt_product(t, dst, dst_off):
                """pt[dst_off half] = x_shift(tap t) * filter(tap t)."""
                ci = 0 if t < 32 else 1
                t0, t1 = chunks[ci]
                dy, dx = t // K, t % K
                xsrc, bc = (xpe, dx) if dx % 2 == 0 else (xpo, dx - 1)
                in0 = ap_of(
                    xsrc, (hp * 2 + dy) * XW + bc,
                    [[GC * CST, 128], [CST, GC], [XW, 2], [1, W]],
                )
                in1 = ap_of(
                    fts[ci], (t - t0) * 2 * W,
                    [[2 * 32 * W, 128], [0, GC], [W, 2], [1, W]],
                )
                pout = ap_of(dst, dst_off, [[2048, 128], [128, GC], [W, 2], [1, W]])
                nc.vector.tensor_tensor(pout, in0, in1, op=MULT)

            # one TT product + 4 accumulate matmuls per tap
            for t in range(NTAP):
                pt = ptp.tile([128, 2048], BF16, tag="pt", name=f"pt_{hp}_{t}")
                tt_product(t, pt, 0)
                for j in range(4):
                    nc.tensor.matmul(
                        pso[:, j * 512:(j + 1) * 512],
                        idbs[:],
                        ap_of(pt, j * 512, [[2048, 128], [1, 512]]),
                        start=(t == 0),
                        stop=(t == NTAP - 1),
                    )
            ob = osb.tile([128, 2048], F32, tag="ob")
            nc.scalar.activation(ob[:], pso[:], COPY)
            nc.sync.dma_start(
                ap_of(out, hp * 2048, [[8 * 2048, 128], [1, 2048]]), ob[:]
            )


def prep_weights(inputs):
    """Host-side weight transforms shared by all cores."""
    w1 = np.asarray(inputs["w1"], np.float32)   # [64, 512, 3, 3]
    w2 = np.asarray(inputs["w2"], np.float32)
    w3 = np.asarray(inputs["w3"], np.float32)   # [512, 64, 3, 3]
    dck_w1 = np.asarray(inputs["dck_w1"], np.float32)  # [128, 512, 1, 1]
    dck_w2 = np.asarray(inputs["dck_w2"], np.float32)  # [1568, 128, 1, 1]

    def tapify(w):  # [co, ci, 3, 3] -> [9, ci, co]
        return np.ascontiguousarray(w.transpose(2, 3, 1, 0).reshape(9, w.shape[1], w.shape[0]))

    w1sb = tapify(w1).reshape(9, 4, 128, HID).transpose(2, 0, 1, 3).reshape(128, 9 * 4 * HID)
    w2sb = tapify(w2).transpose(1, 0, 2).reshape(HID, 9 * HID)
    w3sb = tapify(w3).transpose(1, 0, 2).reshape(HID, 9 * C)
    dw1sb = dck_w1.reshape(RED, C).T.reshape(4, 128, RED).transpose(1, 0, 2).reshape(128, 4 * RED)

    bn_g = np.asarray(inputs["bn_gamma"], np.float32)
    bn_b = np.asarray(inputs["bn_beta"], np.float32)
    bn_m = np.asarray(inputs["bn_mean"], np.float32)
    bn_v = np.asarray(inputs["bn_var"], np.float32)
    inv_std = bn_g / np.sqrt(bn_v + 1e-5)
    shift = bn_b - bn_m * inv_std

    dw2 = dck_w2.reshape(G, NTAP, RED)          # [g, t, red]
    dw2p = np.zeros((G, TPAD, RED), np.float32)
    dw2p[:, :NTAP] = dw2
    dw2t = np.ascontiguousarray(dw2p.reshape(G * TPAD, RED).T)  # [red, g*64]

    return {
        "w1t": np.ascontiguousarray(w1sb),
        "b1": np.asarray(inputs["b1"], np.float32).reshape(HID, 1),
        "w2t": np.ascontiguousarray(w2sb),
        "b2": np.asarray(inputs["b2"], np.float32).reshape(HID, 1),
        "w3t": np.ascontiguousarray(w3sb),
        "b3": np.ascontiguousarray(np.asarray(inputs["b3"], np.float32).reshape(4, 128).T),
        "dw1t": dw1sb.astype(ml_dtypes.bfloat16),
        "bnsc": inv_std.reshape(RED, 1),
        "bnsh": shift.reshape(RED, 1),
        "dw2t": dw2t.astype(ml_dtypes.bfloat16),
        "idb": np.eye(128).astype(ml_dtypes.bfloat16),
    }


def prep_x(xi):
    """Per-sample input layouts: xcr (conv, pad 1), xpe/xpo (apply, pad 3 + shift)."""
    x = np.asarray(xi, np.float32)
    xcr = np.pad(x, ((0, 0), (1, 1), (1, 1))).reshape(C, PHW)
    xp3 = np.pad(x, ((0, 0), (3, 3), (3, 3))).astype(ml_dtypes.bfloat16)  # [512,70,70]
    xps = np.zeros_like(xp3)
    xps[:, :, :-1] = xp3[:, :, 1:]
    def bands(a):  # [512,70,70] -> [128=(band,g), GC*CST]
        v = a.reshape(G, GC, H + 6, XW)
        o = np.empty((NB, G, GC, XH, XW), ml_dtypes.bfloat16)
        for b in range(NB):
            o[b] = v[:, :, b * BH:b * BH + XH]
        return o.reshape(128, GC * CST)
    return {
        "xcr": np.ascontiguousarray(xcr),
        "xpe": np.ascontiguousarray(bands(xp3)),
        "xpo": np.ascontiguousarray(bands(xps)),
    }


def unpack_out(raw, x):
    """[128, 8*2048] filter-term -> [C,H,W], plus residual x."""
    r = np.asarray(raw, np.float32).reshape(NB, G, 8, GC, 2, W)
    r = r.transpose(1, 3, 0, 2, 4, 5).reshape(C, H, W)
    return r + np.asarray(x, np.float32)


_NC_CACHE = {}


def get_nc():
    if "nc" not in _NC_CACHE:
        _NC_CACHE["nc"] = build_nc()
    return _NC_CACHE["nc"]


def make_in_maps(inputs):
    wmap = prep_weights(inputs)
    x = np.asarray(inputs["x"], np.float32)
    return [{**prep_x(x[i]), **wmap} for i in range(N_CORES)]


def kernel(**inputs):
    nc = get_nc()
    x = np.asarray(inputs["x"], np.float32)
    in_maps = make_in_maps(inputs)
    res = bass_utils.run_bass_kernel_spmd(nc, in_maps, core_ids=list(range(N_CORES)))
    return np.stack(
        [unpack_out(res.results[i]["out"], x[i]) for i in range(N_CORES)]
    ).astype(np.float32)
